# revision 2
# baseline (speedup 1.0000x reference)
"""Single-head causal attention (B=16, T=2048, C=1024, D=128) on 8 TRN2 cores.

Data-parallel over batch: each core handles 2 batches. Inside each core:
  xT = transpose(x) via PE transposes           [C on partitions]
  qT/kT/vT = W.T @ xT  (fp32r matmuls)          [D on partitions, T free]
  V = transpose(vT)                             [Tk on partitions, D free]
  per 512-wide query block, per 128-key tile:
    S^T tile = kT_tile.T @ qT_block             [Tk part, Tq free] (PSUM)
    + causal mask on diagonal tiles (DVE add)
    P^T = exp(scale * S^T)  (ACT, writes f32r SBUF)
    OT  += V_tile.T @ P^T                       [D part, Tq free]
    rsum += ones.T @ P^T                        [1, Tq]
  rsumT = tiny transpose matmuls -> [Tq part, 1] ; recip on DVE
  O = transpose(OT) normalized during PSUM evac by recipT (tensor_scalar_mul)
"""

import numpy as np

import concourse.bacc as bacc
import concourse.mybir as mybir
import concourse.tile as tile
from concourse.bass_utils import run_bass_kernel_spmd
from concourse.masks import make_identity

F32 = mybir.dt.float32
F32R = mybir.dt.float32r

B, T, C, D = 16, 2048, 1024, 128
NCORES = 8
BLOC = B // NCORES  # batches per core
NBLK = T // 512  # query blocks of width 512
NKT = T // 128  # key tiles of 128
SCALE = float(D) ** -0.5


def _build():
    nc = bacc.Bacc("TRN2", target_bir_lowering=False, debug=False, num_devices=NCORES)
    x_d = nc.dram_tensor("x", [BLOC, T, C], F32, kind="ExternalInput").ap()
    wq_d = nc.dram_tensor("Wq", [C, D], F32, kind="ExternalInput").ap()
    wk_d = nc.dram_tensor("Wk", [C, D], F32, kind="ExternalInput").ap()
    wv_d = nc.dram_tensor("Wv", [C, D], F32, kind="ExternalInput").ap()
    out_d = nc.dram_tensor("out", [BLOC, T, D], F32, kind="ExternalOutput").ap()

    with tile.TileContext(nc) as tc:
        _emit(nc, tc, x_d, (wq_d, wk_d, wv_d), out_d)
    nc.compile()
    return nc


def _emit(nc, tc, x_d, w_ds, out_d):
    from contextlib import ExitStack

    ctx = ExitStack()
    with ctx:
        const = ctx.enter_context(tc.tile_pool(name="const", bufs=1))
        xtp = ctx.enter_context(tc.tile_pool(name="xtp", bufs=1))
        stage = ctx.enter_context(tc.tile_pool(name="stage", bufs=3))
        qkv = ctx.enter_context(tc.tile_pool(name="qkv", bufs=2))
        ppool = ctx.enter_context(tc.tile_pool(name="ppool", bufs=6))
        small = ctx.enter_context(tc.tile_pool(name="small", bufs=2))
        ps_t = ctx.enter_context(tc.tile_pool(name="ps_t", bufs=1, space="PSUM"))
        ps_proj = ctx.enter_context(tc.tile_pool(name="ps_proj", bufs=2, space="PSUM"))
        ps_s = ctx.enter_context(tc.tile_pool(name="ps_s", bufs=2, space="PSUM"))
        ps_ot = ctx.enter_context(tc.tile_pool(name="ps_ot", bufs=1, space="PSUM"))
        ps_rs = ctx.enter_context(tc.tile_pool(name="ps_rs", bufs=2, space="PSUM"))

        # ---- constants ----
        ident = const.tile([128, 128], F32, tag="ident")
        make_identity(nc, ident)
        masks = const.tile([128, 4, 512], F32, tag="masks")
        nc.gpsimd.memset(masks, 0.0)
        for i in range(4):
            # valid (keep 0.0) iff q - k - 128*i >= 0 ; else fill -1e30
            nc.gpsimd.affine_select(
                out=masks[:, i, :],
                in_=masks[:, i, :],
                compare_op=mybir.AluOpType.is_ge,
                fill=-1e30,
                base=-128 * i,
                pattern=[[1, 512]],
                channel_multiplier=-1,
            )
        ones_f32 = const.tile([128, 1], F32, tag="ones_f32")
        nc.vector.memset(ones_f32, 1.0)
        ones_col = const.tile([128, 1], F32R, tag="ones")
        nc.vector.tensor_copy(ones_col, ones_f32)
        one_one = const.tile([1, 1], F32, tag="oneone")
        nc.vector.memset(one_one, 1.0)
        w_t = const.tile([128, 3, 8, 128], F32R, tag="w")
        for p in range(3):
            nc.sync.dma_start(
                out=w_t[:, p],
                in_=w_ds[p].bitcast(F32R).rearrange("(k p) d -> p k d", p=128),
            )

        # evac engine round-robin (PSUM -> SBUF copies)
        evac_state = [0]

        def evac(out_ap, in_ap):
            if evac_state[0] % 2 == 0:
                nc.vector.tensor_copy(out_ap, in_ap)
            else:
                nc.scalar.copy(out_ap, in_ap)
            evac_state[0] += 1

        for b in range(BLOC):
            # ---- phase X: load x and transpose to xT [C-part, T] ----
            xT = xtp.tile([128, 8, T], F32R, tag="xT")
            for g in range(T // 512):
                for cc in range(8):
                    st = stage.tile([128, 4, 128], F32, tag="stage")
                    nc.sync.dma_start(
                        out=st,
                        in_=x_d[
                            b, 512 * g : 512 * (g + 1), 128 * cc : 128 * (cc + 1)
                        ].rearrange("(ts p) c -> p ts c", p=128),
                    )
                    tp = ps_t.tile([128, 512], F32, tag="tpose")
                    for ts in range(4):
                        nc.tensor.transpose(
                            tp[:, 128 * ts : 128 * (ts + 1)], st[:, ts, :], ident
                        )
                    evac(xT[:, cc, 512 * g : 512 * (g + 1)], tp)

            # ---- phase P: projections qT/kT [D-part, T], V [Tk-part, D] ----
            qT = qkv.tile([128, T], F32R, tag="qT")
            kT = qkv.tile([128, T], F32R, tag="kT")
            V = qkv.tile([128, NKT, 128], F32R, tag="V")
            for j in range(NBLK):
                sl = slice(512 * j, 512 * (j + 1))
                for p, dst in ((0, qT), (1, kT), (2, None)):
                    acc = ps_proj.tile([128, 512], F32, tag="proj")
                    for kk in range(8):
                        nc.tensor.matmul(
                            acc,
                            w_t[:, p, kk],
                            xT[:, kk, sl],
                            start=(kk == 0),
                            stop=(kk == 7),
                        )
                    if dst is not None:
                        evac(dst[:, sl], acc)
                    else:
                        vt_tmp = small.tile([128, 512], F32, tag="vt")
                        evac(vt_tmp, acc)
                        vp = ps_t.tile([128, 512], F32, tag="tpose")
                        for m in range(4):
                            nc.tensor.transpose(
                                vp[:, 128 * m : 128 * (m + 1)],
                                vt_tmp[:, 128 * m : 128 * (m + 1)],
                                ident,
                            )
                        evac(V[:, 4 * j : 4 * (j + 1)].rearrange("p m d -> p (m d)"), vp)

            # ---- phase A: attention ----
            for j in range(NBLK):
                sl = slice(512 * j, 512 * (j + 1))
                ntk = 4 * (j + 1)
                ot = ps_ot.tile([128, 512], F32, tag="ot")
                rs = ps_rs.tile([1, 512], F32, tag="rs")
                for tk in range(ntk):
                    sp = ps_s.tile([128, 512], F32, tag="s")
                    nc.tensor.matmul(
                        sp,
                        kT[:, 128 * tk : 128 * (tk + 1)],
                        qT[:, sl],
                        start=True,
                        stop=True,
                    )
                    if tk >= 4 * j:
                        nc.vector.tensor_tensor(
                            sp, sp, masks[:, tk - 4 * j, :], mybir.AluOpType.add
                        )
                    pt = ppool.tile([128, 512], F32R, tag="p")
                    nc.scalar.activation(
                        pt, sp, mybir.ActivationFunctionType.Exp, scale=SCALE
                    )
                    nc.tensor.matmul(
                        ot, V[:, tk], pt, start=(tk == 0), stop=(tk == ntk - 1)
                    )
                    nc.tensor.matmul(
                        rs, ones_col, pt, start=(tk == 0), stop=(tk == ntk - 1)
                    )
                # rowsum -> transposed reciprocal
                rs_sb = small.tile([1, 512], F32, tag="rssb")
                nc.vector.tensor_copy(rs_sb, rs)
                rsT = ps_rs.tile([128, 4], F32, tag="rs")
                for t in range(4):
                    nc.tensor.matmul(
                        rsT[:, t : t + 1],
                        rs_sb[0:1, 128 * t : 128 * (t + 1)],
                        one_one,
                        start=True,
                        stop=True,
                    )
                recipT = small.tile([128, 4], F32, tag="recip")
                nc.vector.reciprocal(recipT, rsT)
                # OT -> SBUF, transpose to natural layout, normalize, DMA out
                ot_sb = small.tile([128, 512], F32, tag="otsb")
                evac(ot_sb, ot)
                op = ps_t.tile([128, 512], F32, tag="tpose")
                for t in range(4):
                    nc.tensor.transpose(
                        op[:, 128 * t : 128 * (t + 1)],
                        ot_sb[:, 128 * t : 128 * (t + 1)],
                        ident,
                    )
                o_sb = small.tile([128, 4, 128], F32, tag="osb")
                for t in range(4):
                    nc.vector.tensor_scalar_mul(
                        o_sb[:, t, :],
                        op[:, 128 * t : 128 * (t + 1)],
                        recipT[:, t : t + 1],
                    )
                nc.sync.dma_start(
                    out=out_d[b, sl, :].rearrange("(t p) d -> p t d", p=128),
                    in_=o_sb,
                )


_NC = None


def _get_nc():
    global _NC
    if _NC is None:
        _NC = _build()
    return _NC


def kernel(x, Wq, Wk, Wv):
    nc = _get_nc()
    x = np.ascontiguousarray(x, dtype=np.float32)
    in_maps = [
        {"x": x[BLOC * c : BLOC * (c + 1)], "Wq": Wq, "Wk": Wk, "Wv": Wv}
        for c in range(NCORES)
    ]
    res = run_bass_kernel_spmd(nc, in_maps, core_ids=list(range(NCORES)))
    return np.concatenate([res.results[c]["out"] for c in range(NCORES)], axis=0)
